# revision 28
# baseline (speedup 1.0000x reference)
"""Trainium2 Bass kernel for nn_MultiHeadAttention_7954279432294.

Reference computation (per batch b, row h):
    qp = q^T Wq^T + bq       [W, C]   (1x1 conv channel mixing)
    kp = k^T Wk^T + bk       [W, C]
    vp = v^T Wv^T + bv       [W, C]
    out = (qp @ kp^T) @ vp   [W, C]   (linear attention, NO softmax)
    result = out^T + q       [C, W]   (NCHW + residual)

Key optimization: no softmax => reassociate (qp @ kp^T) @ vp = qp @ (kp^T @ vp),
where S = kp^T @ vp is only [C, C] = [64, 64]. This is an 8x FLOP reduction vs
materializing the [512, 512] attention matrix.

Sharding: data-parallel over B (8 batches over 8 cores), weights replicated,
no cross-device communication.

Per-core layout trick: H=64 rows are processed as 32 pairs (h, h+8) packed
into the 128 SBUF partitions (channels 0:64 = h, 64:128 = h+8), so every DMA
uses all 128 partitions and every DVE/ACT op runs at full width.
"""

import numpy as np

import concourse.bass as bass
import concourse.mybir as mybir
import concourse.tile as tile
from concourse.bass_utils import run_bass_kernel_spmd

B, C, H, W = 8, 64, 64, 512
HW = H * W
F32 = mybir.dt.float32

# chunking: 4 chunks of 16 h-rows; each chunk tile is [128, 8*512] = 2 MB
N_CHUNK = 4
H_PER_CHUNK = H // N_CHUNK          # 16
PAIRS_PER_CHUNK = H_PER_CHUNK // 2  # 8
CHUNK_F = PAIRS_PER_CHUNK * W       # 4096


def _add_bcast(nc, out_ap, in0_ap, bias_tile, reps, width=C):
    """out = in0 + bias, where bias is a [128, width] tile broadcast `reps`
    times along the free dim (out/in0 are [128, reps*width])."""
    out3 = out_ap.rearrange("p (r c) -> p r c", c=width)
    in03 = in0_ap.rearrange("p (r c) -> p r c", c=width)
    b2 = bias_tile[:, :]
    bias3 = bass.AP(
        tensor=b2.tensor,
        offset=b2.offset,
        ap=[b2.ap[0], [0, reps], b2.ap[1]],
    )
    nc.vector.tensor_tensor(
        out=out3, in0=in03, in1=bias3, op=mybir.AluOpType.add
    )


def build_nc(hw_workaround: bool = False, reps: int = 1) -> bass.Bass:
    """reps>1 repeats the whole computation inside the NEFF (idempotent) —
    used only for differential HW timing (launch overhead cancels)."""
    nc = bass.Bass()

    # weights are preprocessed host-side in kernel():
    #   Wq -> Wq^T [i, o] duplicated on both partition halves -> [128, C]
    #   Wkv -> block-diag [[Wk^T, 0], [0, Wv^T]] -> [128, 128]
    #   bq -> per-partition column duplicated -> [128, 1]
    #   bkv -> every partition = concat(bk, bv) -> [128, 128]
    q_d = nc.declare_dram_parameter("q", [C, HW], F32, isOutput=False)
    k_d = nc.declare_dram_parameter("k", [C, HW], F32, isOutput=False)
    v_d = nc.declare_dram_parameter("v", [C, HW], F32, isOutput=False)
    Wq_d = nc.declare_dram_parameter("Wq", [128, C], F32, isOutput=False)
    Wkv_d = nc.declare_dram_parameter("Wkv", [128, 128], F32, isOutput=False)
    bq_d = nc.declare_dram_parameter("bq", [128, 1], F32, isOutput=False)
    bkv_d = nc.declare_dram_parameter("bkv", [128, 128], F32, isOutput=False)
    out_d = nc.declare_dram_parameter("out", [C, HW], F32, isOutput=True)

    # chunk ch, g-half: DRAM region q[c, ch*8192 + g*4096 + e] maps to SBUF
    # partitions g*64+c. One [64, 4096] DMA per (tensor, chunk, half).
    def dram_half(d, ch, g):
        lo = ch * 2 * CHUNK_F + g * CHUNK_F
        return d[:, lo : lo + CHUNK_F]

    with tile.TileContext(nc) as tc:
        with (
            tc.tile_pool(name="const", bufs=1) as const,
            tc.tile_pool(name="io", bufs=2) as io,
            tc.tile_pool(name="mid", bufs=2) as mid,
            tc.tile_pool(name="ps2", bufs=2, space="PSUM") as ps2,
            tc.tile_pool(name="ps1", bufs=1, space="PSUM") as ps1,
        ):
            # ---------------- setup: plain DMAs (host did the prep) ----------------
            wTq = const.tile([128, C], F32)
            nc.sync.dma_start(out=wTq[:, :], in_=Wq_d[:, :])

            wkv = const.tile([128, 128], F32)
            nc.sync.dma_start(out=wkv[:, :], in_=Wkv_d[:, :])

            bq2 = const.tile([128, 1], F32)
            nc.sync.dma_start(out=bq2[:, :], in_=bq_d[:, :])

            bkv = const.tile([128, 128], F32)
            nc.sync.dma_start(out=bkv[:, :], in_=bkv_d[:, :])

            # ---------------- main loop ----------------
            # HW constraints (found empirically on this device):
            #   - consecutive matmuls may NOT switch tile_position rows unless
            #     row == col ("diagonal"); column switches are fine.
            #   Safe configs used here: (0, x) for any x, and (64, 64).
            for ch in [c for _ in range(reps) for c in range(N_CHUNK)]:
                q_sb = io.tile([128, CHUNK_F], F32, tag="q_sb")
                # kv{g}_sb rows 0:64 = k channels, 64:128 = v channels (h-group g)
                kv0_sb = io.tile([128, CHUNK_F], F32, tag="kv0_sb")
                kv1_sb = io.tile([128, CHUNK_F], F32, tag="kv1_sb")
                o_sb = io.tile([128, CHUNK_F], F32, tag="o_sb")
                lo, hi = slice(0, C), slice(C, 128)
                for g, kv_sb in ((0, kv0_sb), (1, kv1_sb)):
                    nc.sync.dma_start(out=kv_sb[lo, :], in_=dram_half(k_d, ch, g))
                    nc.sync.dma_start(out=kv_sb[hi, :], in_=dram_half(v_d, ch, g))
                    gp = slice(g * C, (g + 1) * C)
                    nc.sync.dma_start(out=q_sb[gp, :], in_=dram_half(q_d, ch, g))

                for hp in range(PAIRS_PER_CHUNK):
                    hs = slice(hp * W, (hp + 1) * W)

                    # --- fused kp+vp projection into [w, (kp|vp)] layout ---
                    # one matmul per (g, j): lhsT = [k;v] slice [128, 64],
                    # rhs = block-diag Wkv [128, 128] ->
                    # pv_ps[64g + wl, j*128 + (c | 64+c)] = (kp_g | vp_g)
                    pv_ps = ps2.tile([128, 1024], F32, tag="pv_ps")
                    for g, kv_sb in ((0, kv0_sb), (1, kv1_sb)):
                        gp = slice(g * C, (g + 1) * C)
                        for j in range(8):
                            ws = slice(hp * W + j * C, hp * W + (j + 1) * C)
                            nc.tensor.matmul(
                                pv_ps[gp, j * 128 : (j + 1) * 128],
                                kv_sb[:, ws], wkv[:, :],
                                start=True, stop=True,
                            )
                    pv_sb = mid.tile([128, 1024], F32, tag="pv_sb")
                    _add_bcast(nc, pv_sb[:, :], pv_ps[:, :], bkv, 8, width=128)

                    # --- S = kp^T @ vp  [c, c'] per h (8 accumulating K=64 chunks) ---
                    S_ps = ps1.tile([128, C], F32, tag="S_ps")
                    for g in range(2):
                        gp = slice(g * C, (g + 1) * C)
                        for j in range(8):
                            nc.tensor.matmul(
                                S_ps[gp, :],
                                pv_sb[gp, j * 128 : j * 128 + C],
                                pv_sb[gp, j * 128 + C : (j + 1) * 128],
                                start=(j == 0), stop=(j == 7),
                            )
                    S_sb = mid.tile([128, C], F32, tag="S_sb")
                    nc.scalar.copy(S_sb[:, :], S_ps[:, :])

                    # --- qp^T projection [c, w] ---
                    qp_ps = ps1.tile([128, 512], F32, tag="qp_ps")
                    for g in range(2):
                        gp = slice(g * C, (g + 1) * C)
                        nc.tensor.matmul(
                            qp_ps[gp, :], wTq[gp, :], q_sb[gp, hs],
                            start=True, stop=True,
                        )
                    qp_sb = mid.tile([128, 512], F32, tag="qp_sb")
                    nc.scalar.add(qp_sb[:, :], qp_ps[:, :], add=bq2[:, :])

                    # --- out^T[c', w] = sum_c S[c,c'] qp^T[c,w] ---
                    out_ps = ps2.tile([128, 512], F32, tag="out_ps")
                    for g in range(2):
                        gp = slice(g * C, (g + 1) * C)
                        nc.tensor.matmul(
                            out_ps[gp, :], S_sb[gp, :], qp_sb[gp, :],
                            start=True, stop=True,
                        )

                    # --- residual add, write into the output chunk tile ---
                    nc.vector.tensor_tensor(
                        out=o_sb[:, hs], in0=out_ps[:, :], in1=q_sb[:, hs],
                        op=mybir.AluOpType.add,
                    )

                for g in range(2):
                    gp = slice(g * C, (g + 1) * C)
                    nc.sync.dma_start(out=dram_half(out_d, ch, g), in_=o_sb[gp, :])

    if hw_workaround:
        _absorb_matmul_waits(nc)
    nc.finalize()
    return nc


def _absorb_matmul_waits(nc):
    """This walrus build rejects any engine instruction carrying more than one
    sync wait. Split an instruction's n waits into n same-engine NoOps (one
    wait each) inserted right before it: engines execute their stream in FIFO
    order, so the instruction stays correctly gated."""
    ctr = 0
    for bb in nc.m.functions[0].blocks:
        insts = bb.instructions
        i = 0
        while i < len(insts):
            inst = insts[i]
            si = inst.sync_info
            if si is not None and si.on_wait and len(si.on_wait) > 1:
                for w in si.on_wait:
                    nop = mybir.InstNoOp(
                        name=f"I-mmwait-{ctr}", engine=inst.engine, ins=[], outs=[]
                    )
                    ctr += 1
                    nop.sync_info = mybir.SyncInfo(on_wait=[w], on_update=[])
                    insts.insert(i, nop)
                    i += 1
                inst.sync_info = mybir.SyncInfo(
                    on_wait=[], on_update=list(si.on_update)
                )
            i += 1


_NC_CACHE = None
_RUN_KWARGS = {}   # test harness can set e.g. {"trace": True}
LAST_RESULT = None  # BassKernelResults of the last kernel() call


def _get_nc():
    global _NC_CACHE
    if _NC_CACHE is None:
        # the 1-wait workaround is needed for the HW compile path only;
        # CoreSim/TimelineSim consume a clean build_nc() module.
        _NC_CACHE = build_nc(hw_workaround=True)
    return _NC_CACHE


def prep_params(Wq, bq, Wk, bk, Wv, bv):
    Wq = np.asarray(Wq, dtype=np.float32)
    Wk = np.asarray(Wk, dtype=np.float32)
    Wv = np.asarray(Wv, dtype=np.float32)
    bq = np.asarray(bq, dtype=np.float32).reshape(C)
    bk = np.asarray(bk, dtype=np.float32).reshape(C)
    bv = np.asarray(bv, dtype=np.float32).reshape(C)

    # Wq^T duplicated on both halves -> [128, C]
    Wq_p = np.ascontiguousarray(np.concatenate([Wq.T, Wq.T], axis=0))
    # block-diag [[Wk^T, 0], [0, Wv^T]] -> [128, 128]
    Wkv = np.zeros((128, 128), dtype=np.float32)
    Wkv[0:C, 0:C] = Wk.T
    Wkv[C:128, C:128] = Wv.T
    # bq column duplicated -> [128, 1]
    bq_p = np.ascontiguousarray(np.tile(bq.reshape(C, 1), (2, 1)))
    # every partition = concat(bk, bv) -> [128, 128]
    bkv = np.ascontiguousarray(
        np.tile(np.concatenate([bk, bv]).reshape(1, 128), (128, 1))
    )
    return {"Wq": Wq_p, "Wkv": Wkv, "bq": bq_p, "bkv": bkv}


def kernel(q, k, v, Wq, bq, Wk, bk, Wv, bv):
    q = np.ascontiguousarray(np.asarray(q), dtype=np.float32)
    k = np.ascontiguousarray(np.asarray(k), dtype=np.float32)
    v = np.ascontiguousarray(np.asarray(v), dtype=np.float32)
    params = prep_params(Wq, bq, Wk, bk, Wv, bv)

    nc = _get_nc()
    in_maps = []
    for b in range(B):
        in_maps.append(
            {
                "q": q[b].reshape(C, HW),
                "k": k[b].reshape(C, HW),
                "v": v[b].reshape(C, HW),
                **params,
            }
        )
    res = run_bass_kernel_spmd(nc, in_maps, list(range(B)), **_RUN_KWARGS)
    global LAST_RESULT
    LAST_RESULT = res
    out = np.stack([res.results[b]["out"].reshape(C, H, W) for b in range(B)])
    return out
